# revision 11
# baseline (speedup 1.0000x reference)
"""Deformable conv (3x3, pad 1) on 8 trn2 NeuronCores.

Sharding: batch b = core//2 (B=4), output-row half = core%2.
Per core algorithm (contract-channels-first formulation):
  G^T[p, (k,o)] = sum_c x[c,p] * w_def[o,c,k]   (PE, bf16, window rows only)
  offsets^T[i, 18] = conv(x, w_off) at own positions (PE im2col, bf16)
  bilinear indices/weights (DVE, fp32)
  gather G rows by index (SWDGE dma_gather, 9 taps x 2 y-corners, 1KB rows
  carrying both x-corners); positions land on partitions
  out^T[hw, o] = sum_{k,corner} wgt * G_k[idx]  via PE matmuls with
  diag(wgt) as lhsT accumulating in PSUM.
Host: input padding/layout, final transpose + bias (numpy).
"""

import numpy as np

B, C, O, H, W = 4, 256, 256, 64, 64
K, KK, PAD = 3, 9, 1
HW = H * W
HALF = 32                    # output rows per core
NPOS = HALF * W              # 2048 positions per core
NBLK = NPOS // 128           # 16
WIN = 44                     # gather-window image rows (h0-6 .. h0+37)
XROWS = WIN + 2              # xp rows: h0-7 .. h0+38 (conv + window border)
XCOLS = 66
GROWS = 64 * (WIN + 2)       # 2944 G rows: 1 pad row-block each side
GALLOC = GROWS + 64          # spare zeroed rows for the +1 overlap read
NCH = 2 * KK                 # 18 offset channels

_CACHE = {}


def _build():
    import concourse.bacc as bacc
    import concourse.bass as bass
    import concourse.tile as tile
    import concourse.mybir as mybir

    dt = mybir.dt
    Alu = mybir.AluOpType
    Act = mybir.ActivationFunctionType

    nc = bacc.Bacc(None, target_bir_lowering=False)

    # ---- I/O ----
    xp_d = nc.dram_tensor("xp", [C, XROWS * XCOLS], dt.bfloat16, kind="ExternalInput")
    wrhs_d = nc.dram_tensor("wrhs", [C, KK * O + KK * NCH], dt.bfloat16, kind="ExternalInput")
    tmat_d = nc.dram_tensor("tmat", [128, 18 * 128], dt.bfloat16, kind="ExternalInput")
    sjj_d = nc.dram_tensor("sjj", [128, 8 * 128], dt.float16, kind="ExternalInput")
    id_d = nc.dram_tensor("id128", [128, 128], dt.bfloat16, kind="ExternalInput")
    bpy_d = nc.dram_tensor("bpy", [128, 144], dt.float32, kind="ExternalInput")
    bpx_d = nc.dram_tensor("bpx", [128, 144], dt.float32, kind="ExternalInput")
    clips_d = nc.dram_tensor("clips", [128, 4], dt.float32, kind="ExternalInput")
    cdy_d = nc.dram_tensor("cdy", [128, 2], dt.float32, kind="ExternalInput")
    boff_d = nc.dram_tensor("boff", [128, NCH], dt.float32, kind="ExternalInput")
    out_d = nc.dram_tensor("outT", [NPOS, O], dt.float32, kind="ExternalOutput")

    with tile.TileContext(nc) as tc:
        with (
            tc.tile_pool(name="const", bufs=1) as const,
            tc.tile_pool(name="gsb", bufs=2) as gsb_pool,
            tc.tile_pool(name="math", bufs=1) as math_pool,
            tc.tile_pool(name="gth", bufs=3) as gth_pool,
            tc.tile_pool(name="diag", bufs=72) as diag_pool,
            tc.tile_pool(name="dram", bufs=1, space="DRAM") as dram_pool,
        ):
            # ---- load inputs to SBUF ----
            xp = [const.tile([128, XROWS * XCOLS], dt.bfloat16, tag=f"xp{i}", name=f"xp{i}") for i in range(2)]
            wrhs = [const.tile([128, KK * O + KK * NCH], dt.bfloat16, tag=f"wr{i}", name=f"wr{i}") for i in range(2)]
            for i in range(2):
                nc.sync.dma_start(xp[i][:], xp_d[128 * i:128 * (i + 1), :])
                nc.sync.dma_start(wrhs[i][:], wrhs_d[128 * i:128 * (i + 1), :])
            tmat = const.tile([128, 18 * 128], dt.bfloat16)
            nc.sync.dma_start(tmat[:], tmat_d[:, :])
            sjj = const.tile([128, 8 * 128], dt.float16)
            nc.sync.dma_start(sjj[:], sjj_d[:, :])
            id128 = const.tile([128, 128], dt.bfloat16)
            nc.sync.dma_start(id128[:], id_d[:, :])
            bpy = const.tile([128, 144], dt.float32)
            nc.sync.dma_start(bpy[:], bpy_d[:, :])
            bpx = const.tile([128, 144], dt.float32)
            nc.sync.dma_start(bpx[:], bpx_d[:, :])
            clips = const.tile([128, 4], dt.float32)
            nc.sync.dma_start(clips[:], clips_d[:, :])
            cdy = const.tile([128, 2], dt.float32)
            nc.sync.dma_start(cdy[:], cdy_d[:, :])
            boffb = const.tile([128, NCH], dt.float32)
            nc.sync.dma_start(boffb[:], boff_d[:, :])

            zer = const.tile([128, 2304], dt.bfloat16)
            nc.vector.memset(zer[:], 0)

            G = dram_pool.tile([KK, GALLOC * O], dt.bfloat16)
            Gt = G[:].tensor
            Goff = G[:].offset

            # zero G pad rows: head rows 0..63, tail rows GROWS-64..GALLOC-1
            for k in range(KK):
                nc.sync.dma_start(
                    bass.AP(Gt, Goff + k * GALLOC * O, [[128, 128], [1, 128]]),
                    zer[:, :128],
                )
                nc.sync.dma_start(
                    bass.AP(Gt, Goff + k * GALLOC * O + (GROWS - 64) * O,
                            [[256, 128], [1, 256]]),
                    zer[:, :256],
                )

            xcol = [const.tile([128, WIN * 64], dt.bfloat16, tag=f"xc{i}", name=f"xc{i}")
                    for i in range(2)]
            for i in range(2):
                xa = xp[i][:]
                nc.sync.dma_start(
                    xcol[i][:].rearrange("p (r w) -> p r w", r=WIN),
                    bass.AP(xa.tensor, xa.offset + XCOLS + 1,
                            [[xa.ap[0][0], 128], [XCOLS, WIN], [1, 64]]),
                )

            # ---- G + offsets matmuls (PSUM pools scoped, freed before combine) ----
            with (
                tc.tile_pool(name="psG", bufs=2, space="PSUM") as psG,
                tc.tile_pool(name="psO", bufs=1, space="PSUM") as psO,
                tc.tile_pool(name="psW", bufs=2, space="PSUM") as psW,
            ):
                CHUNKS = [(0, 512), (512, 512), (1024, 512), (1536, 512), (2048, 418)]
                offT = const.tile([128, 22 * KK * NCH], dt.bfloat16)  # (pb, k, ch)
                for pb in range(22):
                    gtile = gsb_pool.tile([128, KK * O], dt.bfloat16, tag="gtile", name="gtile")
                    for (c0, cw) in CHUNKS:
                        ps = psG.tile([128, 512], dt.float32, tag="psG", name="psGt")
                        for cb in range(2):
                            nc.tensor.matmul(ps[:, :cw],
                                             xcol[cb][:, pb * 128:(pb + 1) * 128],
                                             wrhs[cb][:, c0:c0 + cw],
                                             start=(cb == 0), stop=(cb == 1))
                        if c0 < 2048:
                            nc.scalar.activation(gtile[:, c0:c0 + cw], ps[:, :cw], Act.Copy)
                        else:
                            nc.scalar.activation(gtile[:, 2048:2304], ps[:, :256], Act.Copy)
                            nc.scalar.activation(offT[:, pb * 162:(pb + 1) * 162],
                                                 ps[:, 256:418], Act.Copy)
                    # one DMA: [128, (k,o)] -> G[k, 64+128*pb : .. , :]
                    nc.sync.dma_start(
                        bass.AP(Gt, Goff + (64 + 128 * pb) * O,
                                [[O, 128], [GALLOC * O, KK], [1, O]]),
                        gtile[:].rearrange("p (k o) -> p k o", k=KK),
                    )

                # ---- offsets^T: shift-matrix matmuls over per-tap projections ----
                # taps: delta = (ky-1)*64 + (kx-1); base = 384 + delta
                ps_off = psO.tile([128, NBLK * NCH], dt.float32)
                parts = []  # (tmat_idx, tap, block_off)
                ti = 0
                for t in range(KK):
                    ky, kx = t // 3, t % 3
                    base = 384 + (ky - 1) * 64 + (kx - 1)
                    b0, r = base // 128, base % 128
                    parts.append((ti, t, b0))
                    ti += 1
                    if r > 0:
                        parts.append((ti, t, b0 + 1))
                        ti += 1
                NPARTS = len(parts)
                for pi, (tix, t, boff) in enumerate(parts):
                    for blk in range(NBLK):
                        q = blk + boff
                        nc.tensor.matmul(
                            ps_off[:, blk * NCH:(blk + 1) * NCH],
                            tmat[:, tix * 128:(tix + 1) * 128],
                            offT[:].rearrange("p (b k c) -> p b k c", b=22, k=KK)[:, q, t],
                            start=(pi == 0 and blk == 0),
                            stop=(pi == NPARTS - 1 and blk == NBLK - 1),
                            skip_group_check=True)
                off_t = math_pool.tile([128, NBLK * NCH], dt.float32)  # (blk, ch)
                nc.scalar.activation(off_t[:], ps_off[:], Act.Copy)
                o3 = off_t[:].rearrange("p (b c) -> p b c", b=NBLK)
                nc.vector.tensor_tensor(
                    o3, o3, boffb[:].unsqueeze(1).broadcast_to([128, NBLK, NCH]),
                    Alu.add)

                # ---- per-position math (layout [128, (blk,k)] = [128, 144]) ----
                offy = o3[:, :, 0:NCH:2]
                offx = o3[:, :, 1:NCH:2]

                def mt(tag):
                    return math_pool.tile([128, 144], dt.float32, tag=tag, name=tag)

                def v3(ap):  # [128,144] -> [128,16,9]
                    return ap.rearrange("p (b k) -> p b k", b=NBLK)

                pys, pxs = mt("pys"), mt("pxs")
                nc.vector.tensor_tensor(v3(pys[:]), offy, v3(bpy[:]), Alu.add)
                nc.vector.tensor_tensor(v3(pxs[:]), offx, v3(bpx[:]), Alu.add)
                ly, lx = mt("ly"), mt("lx")
                fys, fxs = mt("fys"), mt("fxs")
                icast = math_pool.tile([128, 144], dt.int32, tag="icast", name="icast")
                ctmp = mt("ctmp")
                for (p_, f_, l_) in ((pys, fys, ly), (pxs, fxs, lx)):
                    # floor(p) valid under any int-cast rounding mode:
                    # f = float(int(p)); f -= (f > p); l = p - f
                    nc.vector.tensor_copy(icast[:], p_[:])
                    nc.vector.tensor_copy(f_[:], icast[:])
                    nc.vector.tensor_tensor(ctmp[:], f_[:], p_[:], Alu.is_gt)
                    nc.vector.tensor_tensor(f_[:], f_[:], ctmp[:], Alu.subtract)
                    nc.vector.tensor_tensor(l_[:], p_[:], f_[:], Alu.subtract)

                # validity (0/1) per corner
                vy = [mt(f"vy{d}") for d in range(2)]
                vx = [mt(f"vx{d}") for d in range(2)]
                tmp = mt("vtmp")
                for d in range(2):
                    nc.vector.tensor_scalar(vy[d][:], fys[:], 16.0 - d, None, Alu.is_ge)
                    nc.vector.tensor_scalar(tmp[:], fys[:], 80.0 - d, None, Alu.is_lt)
                    nc.vector.tensor_tensor(vy[d][:], vy[d][:], tmp[:], Alu.mult)
                    nc.vector.tensor_scalar(vx[d][:], fxs[:], 16.0 - d, None, Alu.is_ge)
                    nc.vector.tensor_scalar(tmp[:], fxs[:], 80.0 - d, None, Alu.is_lt)
                    nc.vector.tensor_tensor(vx[d][:], vx[d][:], tmp[:], Alu.mult)

                # corner weights wy{0,1}, wx{0,1}
                wy = [mt(f"wy{d}") for d in range(2)]
                wx = [mt(f"wx{d}") for d in range(2)]
                nc.vector.tensor_scalar(wy[0][:], ly[:], -1.0, 1.0, Alu.mult, Alu.add)
                nc.vector.tensor_tensor(wy[0][:], wy[0][:], vy[0][:], Alu.mult)
                nc.vector.tensor_tensor(wy[1][:], ly[:], vy[1][:], Alu.mult)
                nc.vector.tensor_scalar(wx[0][:], lx[:], -1.0, 1.0, Alu.mult, Alu.add)
                nc.vector.tensor_tensor(wx[0][:], wx[0][:], vx[0][:], Alu.mult)
                nc.vector.tensor_tensor(wx[1][:], lx[:], vx[1][:], Alu.mult)

                # W[dy][x01] fp32 (tensor_scalar scalar operand must be fp32)
                Wbf = [[math_pool.tile([128, 144], dt.float32, tag=f"W{dy}{x}", name=f"W{dy}{x}")
                        for x in range(2)] for dy in range(2)]
                for dy in range(2):
                    for x in range(2):
                        nc.vector.tensor_tensor(Wbf[dy][x][:], wy[dy][:], wx[x][:], Alu.mult)

                # clipped coords u0,u1 (y per dy) and v (x), packed fp16
                rhs_pack = math_pool.tile([128, 3 * 144], dt.float16)
                utmp = mt("utmp")
                for d in range(2):
                    nc.vector.tensor_scalar(utmp[:], fys[:], clips[:, 2 * d:2 * d + 1],
                                            clips[:, 2 * d + 1:2 * d + 2], Alu.max, Alu.min)
                    nc.vector.tensor_copy(rhs_pack[:, d * 144:(d + 1) * 144], utmp[:])
                nc.vector.tensor_scalar(utmp[:], fxs[:], 15.0, 79.0, Alu.max, Alu.min)
                nc.vector.tensor_copy(rhs_pack[:, 288:432], utmp[:])

                # ---- wrap relayout via 8 selector matmuls ----
                uvw = math_pool.tile([128, 3 * KK * 128], dt.float32)  # (q, k, j)
                uvw_ap = uvw[:]
                UPITCH = uvw_ap.ap[0][0]
                for jj in range(8):
                    psw = psW.tile([128, 432], dt.float32, tag="psw", name="psw")
                    nc.tensor.matmul(psw[:], sjj[:, jj * 128:(jj + 1) * 128],
                                     rhs_pack[:], start=True, stop=True)
                    src = psw[:].rearrange("p (q b k) -> p q k b", q=3, b=NBLK)
                    dst = bass.AP(uvw_ap.tensor, uvw_ap.offset + jj,
                                  [[UPITCH, 128], [KK * 128, 3], [128, KK], [8, NBLK]])
                    nc.scalar.activation(dst, src, Act.Copy)

            uvw3 = uvw[:].rearrange("p (q f) -> p q f", q=3)
            idxw = math_pool.tile([128, 2 * KK * 128], dt.int16)  # (dy, k, j)
            idxf = math_pool.tile([128, KK * 128], dt.float32, tag="idxf", name="idxf")
            for dy in range(2):
                nc.vector.tensor_scalar(idxf[:], uvw3[:, dy], 64.0, cdy[:, dy:dy + 1],
                                        Alu.mult, Alu.add)
                nc.vector.tensor_tensor(idxf[:], idxf[:], uvw3[:, 2], Alu.add)
                nc.vector.tensor_copy(idxw[:, dy * KK * 128:(dy + 1) * KK * 128], idxf[:])

            # ---- gather + diag-weight + combine matmuls ----
            with tc.tile_pool(name="psA", bufs=1, space="PSUM") as psA:
                acc = [psA.tile([128, 512], dt.float32, tag=f"acc{i}", name=f"acc{i}") for i in range(8)]
                n_kd = 0
                for k in range(KK):
                    for dy in range(2):
                        first = n_kd == 0
                        last = n_kd == 17
                        n_kd += 1
                        gk = gth_pool.tile([128, NBLK * 512], dt.bfloat16, tag="gth", name="gth")
                        in_ap = bass.AP(Gt, Goff + k * GALLOC * O,
                                        [[O, GROWS], [1, 512]])
                        nc.gpsimd.dma_gather(
                            gk[:].rearrange("p (b e) -> p b e", b=NBLK),
                            in_ap,
                            idxw[:, (dy * KK + k) * 128:(dy * KK + k + 1) * 128],
                            num_idxs=NPOS, num_idxs_reg=NPOS,
                            elem_size=512, elem_step=O,
                            single_packet=False,
                        )
                        g3 = gk[:].rearrange("p (b x o) -> p b x o", b=NBLK, x=2)
                        for blk in range(NBLK):
                            for x in range(2):
                                dg = diag_pool.tile([128, 128], dt.bfloat16, tag="dg", name="dg")
                                eng = nc.vector if (blk % 2 == 0) else nc.gpsimd
                                eng.tensor_scalar(
                                    dg[:], id128[:],
                                    Wbf[dy][x][:, blk * KK + k:blk * KK + k + 1],
                                    None, Alu.mult)
                                nc.tensor.matmul(
                                    acc[blk // 2][:, (blk % 2) * 256:(blk % 2) * 256 + 256],
                                    dg[:], g3[:, blk, x],
                                    start=(first and x == 0 and blk % 2 == 0),
                                    stop=(last and x == 1),
                                    skip_group_check=True)

                # ---- evac + store ----
                osb = math_pool.tile([128, NBLK * 256], dt.float32, tag="osb", name="osb")
                for i in range(8):
                    nc.scalar.activation(osb[:, i * 512:(i + 1) * 512], acc[i][:], Act.Copy)
            nc.sync.dma_start(
                bass.AP(out_d, 0, [[O, 128], [128 * O, NBLK], [1, O]]),
                osb[:].rearrange("p (b o) -> p b o", b=NBLK),
            )

    nc.finalize()
    return nc


def _host_prep(x, w_off, b_off, w_def, b_def):
    """Build per-core input maps."""
    import ml_dtypes
    bf16 = ml_dtypes.bfloat16
    x = np.asarray(x, np.float32)
    w_off = np.asarray(w_off, np.float32)
    b_off = np.asarray(b_off, np.float32)
    w_def = np.asarray(w_def, np.float32)
    in_maps = []
    wrhs = np.zeros((C, KK * O + KK * NCH), np.float32)
    wrhs[:, :KK * O] = w_def.reshape(O, C, KK).transpose(1, 2, 0).reshape(C, KK * O)
    # per-tap w_off projection columns: col 2304 + k*18 + ch = w_off[ch, c, ky, kx]
    wrhs[:, KK * O:] = w_off.reshape(NCH, C, KK).transpose(1, 2, 0).reshape(C, KK * NCH)
    wrhs = np.ascontiguousarray(wrhs).astype(bf16)
    # shift matrices for the 9-tap offset sum
    tmat = np.zeros((128, 18, 128), np.float32)
    ti = 0
    for t in range(KK):
        ky, kx = t // 3, t % 3
        base = 384 + (ky - 1) * 64 + (kx - 1)
        r = base % 128
        pd = np.arange(128)
        xm = ((pd % 64) + kx - 1 >= 0) & ((pd % 64) + kx - 1 < 64)
        okA = (pd + r <= 127) & xm
        tmat[(pd + r).clip(0, 127)[okA], ti, pd[okA]] = 1.0
        ti += 1
        if r > 0:
            okB = (pd + r - 128 >= 0) & xm
            tmat[(pd + r - 128).clip(0, 127)[okB], ti, pd[okB]] = 1.0
            ti += 1
    tmat = np.ascontiguousarray(tmat.reshape(128, 18 * 128)).astype(bf16)
    sjj = np.zeros((128, 8, 128), np.float16)
    for jj in range(8):
        for q in range(128):
            sjj[jj * 16 + (q % 16), jj, q] = 1.0
    sjj = np.ascontiguousarray(sjj.reshape(128, 8 * 128))
    id128 = np.eye(128, dtype=np.float32).astype(bf16)
    boff_bc = np.ascontiguousarray(np.tile(b_off[None, :], (128, 1)).astype(np.float32))

    kyv = (np.arange(KK) // 3).astype(np.float32)
    kxv = (np.arange(KK) % 3).astype(np.float32)

    for core in range(8):
        b, half = core // 2, core % 2
        h0 = half * HALF
        # xp: rows h0-7 .. h0+38, cols -1..64, zero-padded
        xp = np.zeros((C, XROWS, XCOLS), np.float32)
        r_lo, r_hi = h0 - 7, h0 + 39
        s_lo, s_hi = max(r_lo, 0), min(r_hi, H)
        xp[:, s_lo - r_lo:s_hi - r_lo, 1:65] = x[b][:, s_lo:s_hi, :]
        xp = np.ascontiguousarray(xp.reshape(C, XROWS * XCOLS)).astype(bf16)

        p = np.arange(128)
        blk = np.arange(NBLK)
        hgrid = (h0 + 2 * blk[None, :, None] + (p[:, None, None] // 64)).astype(np.float32)
        wgrid = ((p % 64)[:, None, None] + np.zeros((1, NBLK, KK))).astype(np.float32)
        bpy = (hgrid + kyv[None, None, :] - 1 + 16).astype(np.float32).reshape(128, 144)
        bpx = (wgrid + kxv[None, None, :] - 1 + 16).astype(np.float32).reshape(128, 144)

        clips = np.zeros((128, 4), np.float32)
        cdy = np.zeros((128, 2), np.float32)
        for d in range(2):
            lo = h0 + 10 - d
            clips[:, 2 * d] = lo
            clips[:, 2 * d + 1] = lo + WIN - 1
            cdy[:, d] = 64.0 * (1 - lo) - 16.0
        in_maps.append({
            "xp": xp, "wrhs": wrhs, "tmat": tmat, "sjj": sjj, "id128": id128,
            "bpy": np.ascontiguousarray(bpy), "bpx": np.ascontiguousarray(bpx),
            "clips": clips, "cdy": cdy, "boff": boff_bc,
        })
    return in_maps


def run_full(inputs, trace=False):
    from concourse.bass_utils import run_bass_kernel_spmd
    if "nc" not in _CACHE:
        _CACHE["nc"] = _build()
    nc = _CACHE["nc"]
    in_maps = _host_prep(inputs["x"], inputs["w_off"], inputs["b_off"],
                         inputs["w_def"], inputs["b_def"])
    res = run_bass_kernel_spmd(nc, in_maps, core_ids=list(range(8)), trace=trace)
    full = np.zeros((B, O, H, W), np.float32)
    for core in range(8):
        b, half = core // 2, core % 2
        o = np.asarray(res.results[core]["outT"], np.float32)   # [2048, 256]
        full[b, :, half * HALF:(half + 1) * HALF, :] = \
            o.reshape(HALF, W, O).transpose(2, 0, 1)
    full += np.asarray(inputs["b_def"], np.float32)[None, :, None, None]
    return full, res.exec_time_ns


def kernel(**inputs):
    out, _ = run_full(inputs, trace=False)
    return out
